# revision 11
# baseline (speedup 1.0000x reference)
"""Modulated deformable conv (warp-norm softmax weights) on 8 TRN2 NeuronCores.

Sharding: 8 cores = (batch 4) x (image half 2). Each core gets a 96-row band
of x (16-row halo), its 80 output rows' offsets/masks (host-transposed to
pixel-major), and computes out[64, 80, 160] f32.

Per-core device pipeline:
  1. x band f32 -> bf16 (cast DMA); PE-transposes build a row-pair table
     T[e] = [xT[e] | xT[e+160]] (bf16, 256B entries), staged to DRAM.
  2. Pixel-major index/bilinear-weight prep on DVE. Clamping is slot-remapped:
     one int16 entry index + 4 slot weights reproduce the reference's
     independently-clipped, validity-zeroed corners exactly.
  3. idx DRAM round-trip into dma_gather's 16-partition-wrapped layout.
  4. dma_gather (HBM source, elem_step=128 elems: overlapping entries, so one
     512B read = the full 2x2 corner quad; pixel-major output).
  5. DVE: Gw = G * wq (broadcast AP, 2x mode); Gy = Gw[xj=0] + Gw[xj=1].
  6. PE: transpose Gy slices to (k,yj,c)-major; GEMM vs softmaxed weights
     (yj-replicated) accumulating in PSUM; ACT evacuates.
"""
import os
import sys

sys.path.insert(0, "/opt/trn_rl_repo")

import numpy as np
import ml_dtypes

import concourse.bass as bass
import concourse.bacc as bacc
import concourse.mybir as mybir
from concourse.tile import TileContext
from concourse.masks import make_identity
from concourse.bass_utils import run_bass_kernel_spmd

bf16 = ml_dtypes.bfloat16
f32 = mybir.dt.float32
bft = mybir.dt.bfloat16
i16 = mybir.dt.int16

H = W = 160
CIN = OC = 64
K = 3
K2 = 9
BAND = 96
OUT_ROWS = 80
NP = OUT_ROWS * W          # 12800
NCHUNK = NP // 128         # 100
GK = NCHUNK * K2           # 900
NIDX = NP * K2             # 115200
N_ENT = (BAND - 1) * W     # 15200
XCOLS = BAND * W           # 15360
XPAD = 15488               # transpose-friendly padded width
BPC = 4                    # chunks per gather block
NBLK = NCHUNK // BPC       # 25
BIDX = BPC * K2 * 128      # 4608

ALU = mybir.AluOpType

_CACHE = {}
LAST_RESULTS = {}


def ap3(tile_ap, off, dims):
    return bass.AP(tile_ap.tensor, tile_ap.offset + off,
                   [tile_ap.ap[0]] + dims)


def _build_program():
    nc = bacc.Bacc("TRN2", num_devices=8)

    xb_in = nc.dram_tensor("xband", [CIN, XCOLS], f32, kind="ExternalInput")
    offm_in = nc.dram_tensor("offm", [128, NCHUNK * 27], f32, kind="ExternalInput")
    byx_in = nc.dram_tensor("byx", [128, NCHUNK * 18], f32, kind="ExternalInput")
    clampb_in = nc.dram_tensor("clampb", [128, 2], f32, kind="ExternalInput")
    wsm2_in = nc.dram_tensor("wsm2", [128, K2 * OC], bft, kind="ExternalInput")
    out_t = nc.dram_tensor("out", [OC, NP], f32, kind="ExternalOutput")
    t_dram = nc.dram_tensor("tdram", [XPAD + 1, 2 * CIN], bft, kind="Internal")
    idx_dram = nc.dram_tensor("idxdram", [NIDX], i16, kind="Internal")

    with TileContext(nc) as tc:
        with tc.tile_pool(name="const", bufs=1) as cpool:
            ident = cpool.tile([128, 128], bft)
            make_identity(nc, ident[:])
            wsm2 = cpool.tile([128, K2, OC], bft)
            nc.sync.dma_start(wsm2[:], wsm2_in[:])
            clampb = cpool.tile([128, 2], f32)
            nc.sync.dma_start(clampb[:], clampb_in[:])
            wq2 = cpool.tile([128, NCHUNK, 36, 2], bft)
            idx_wrap = cpool.tile([128, NIDX // 16], i16)
            out_sb = cpool.tile([OC, NP], f32)

            # ---------- stage 1: quad table ----------
            with tc.tile_pool(name="tbl", bufs=1) as tp, \
                 tc.tile_pool(name="tpsum", bufs=3, space="PSUM") as tpp:
                xb = tp.tile([CIN, XPAD], bft)
                nc.vector.memset(xb[:, XCOLS:], 0.0)
                nc.gpsimd.dma_start(xb[:, :XCOLS], xb_in[:])   # casts f32->bf16
                tsb = tp.tile([128, 119, 2, CIN], bft)
                for q in range(0, 119, 4):
                    nq = min(4, 119 - q)
                    ps = tpp.tile([128, 4, 2, CIN], bft)
                    for j in range(nq):
                        t = q + j
                        nc.tensor.transpose(
                            ps[:, j, 0, :], xb[:, t * 128:(t + 1) * 128],
                            ident[:CIN, :CIN])
                        nc.tensor.transpose(
                            ps[:, j, 1, :], xb[:, t * 128 + W:t * 128 + W + 128],
                            ident[:CIN, :CIN])
                    nc.scalar.copy(tsb[:, q:q + nq, :, :], ps[:, :nq, :, :])
                nc.sync.dma_start(
                    bass.AP(t_dram, 0,
                            [[2 * CIN, 128], [128 * 2 * CIN, 119],
                             [1, 2 * CIN]]),
                    bass.AP(tsb[:].tensor, tsb[:].offset,
                            [[119 * 2 * CIN, 128], [2 * CIN, 119], [1, 2 * CIN]]),
                )

            # ---------- stage 2: prep ----------
            with tc.tile_pool(name="prep", bufs=1) as pp:
                offm = pp.tile([128, NCHUNK, 27], f32)
                nc.sync.dma_start(offm[:], offm_in[:])
                byx = pp.tile([128, NCHUNK, 18], f32)
                nc.sync.dma_start(byx[:], byx_in[:])

                o_ap = offm[:]
                dyx = ap3(o_ap, 0, [[27, NCHUNK], [1, 18]])
                m_v = ap3(o_ap, 18, [[27, NCHUNK], [1, K2]])

                pyx = pp.tile([128, NCHUNK, 18], f32)
                pyx_v = ap3(pyx[:], 0, [[18, NCHUNK], [1, 18]])
                nc.vector.tensor_tensor(out=pyx_v, in0=dyx,
                                        in1=ap3(byx[:], 0,
                                                [[18, NCHUNK], [1, 18]]),
                                        op=ALU.add)
                yx0 = pp.tile([128, NCHUNK, 18], f32)   # floor + 16 bias
                # floor via round-to-nearest(py - 0.5) using the 2^23 trick;
                # exact-integer py lands on (y0-1, fy=1) which samples
                # identically.
                nc.vector.tensor_scalar(out=yx0[:], in0=pyx[:],
                                        scalar1=float(2 ** 23) - 0.5,
                                        scalar2=float(2 ** 23),
                                        op0=ALU.add, op1=ALU.subtract)
                fyx = pp.tile([128, NCHUNK, 18], f32)
                nc.vector.tensor_tensor(out=fyx[:], in0=pyx[:], in1=yx0[:],
                                        op=ALU.subtract)

                wA = pp.tile([128, 2, NCHUNK, K2], f32)
                wB = pp.tile([128, 2, NCHUNK, K2], f32)
                ecl = pp.tile([128, 2, NCHUNK, K2], f32)
                tmp1 = pp.tile([128, NCHUNK, K2], f32)
                tmp2 = pp.tile([128, NCHUNK, K2], f32)
                tmp3 = pp.tile([128, NCHUNK, K2], f32)
                gv0 = pp.tile([128, NCHUNK, K2], f32)
                gv1 = pp.tile([128, NCHUNK, K2], f32)

                for ax in range(2):
                    v0 = ap3(yx0[:], ax * K2, [[18, NCHUNK], [1, K2]])
                    f0 = ap3(fyx[:], ax * K2, [[18, NCHUNK], [1, K2]])
                    eo = ap3(ecl[:], ax * NCHUNK * K2, [[K2, NCHUNK], [1, K2]])
                    wAo = ap3(wA[:], ax * NCHUNK * K2, [[K2, NCHUNK], [1, K2]])
                    wBo = ap3(wB[:], ax * NCHUNK * K2, [[K2, NCHUNK], [1, K2]])
                    emax = 94.0 if ax == 0 else 158.0
                    # t = yx0 - clamp_bias (per-core AP scalar)
                    nc.vector.tensor_scalar(out=tmp1[:], in0=v0,
                                            scalar1=clampb[:, ax:ax + 1],
                                            scalar2=None, op0=ALU.subtract)
                    nc.vector.tensor_scalar(out=eo, in0=tmp1[:], scalar1=0.0,
                                            scalar2=emax, op0=ALU.max,
                                            op1=ALU.min)
                    nc.vector.tensor_tensor(out=tmp2[:], in0=tmp1[:], in1=eo,
                                            op=ALU.subtract)
                    # validity is implied by the eq-gates: every corner
                    # passing an eq test lies in the valid image range.
                    # gv0 = 1 - f
                    nc.vector.tensor_scalar(out=gv0[:], in0=f0, scalar1=1.0,
                                            scalar2=-1.0, op0=ALU.subtract,
                                            op1=ALU.mult)
                    # wA = (te==0)*(1-f) + (te==-1)*f
                    nc.vector.tensor_scalar(out=tmp1[:], in0=tmp2[:],
                                            scalar1=0.0, scalar2=None,
                                            op0=ALU.is_equal)
                    nc.vector.tensor_tensor(out=tmp1[:], in0=tmp1[:],
                                            in1=gv0[:], op=ALU.mult)
                    nc.vector.tensor_scalar(out=tmp3[:], in0=tmp2[:],
                                            scalar1=-1.0, scalar2=None,
                                            op0=ALU.is_equal)
                    nc.vector.tensor_tensor(out=tmp3[:], in0=tmp3[:],
                                            in1=f0, op=ALU.mult)
                    nc.vector.tensor_tensor(out=wAo, in0=tmp1[:], in1=tmp3[:],
                                            op=ALU.add)
                    # wB = (te==0)*f + (te==1)*(1-f)
                    nc.vector.tensor_scalar(out=tmp1[:], in0=tmp2[:],
                                            scalar1=0.0, scalar2=None,
                                            op0=ALU.is_equal)
                    nc.vector.tensor_tensor(out=tmp1[:], in0=tmp1[:],
                                            in1=f0, op=ALU.mult)
                    nc.vector.tensor_scalar(out=tmp3[:], in0=tmp2[:],
                                            scalar1=1.0, scalar2=None,
                                            op0=ALU.is_equal)
                    nc.vector.tensor_tensor(out=tmp3[:], in0=tmp3[:],
                                            in1=gv0[:], op=ALU.mult)
                    nc.vector.tensor_tensor(out=wBo, in0=tmp1[:], in1=tmp3[:],
                                            op=ALU.add)

                # fold mask into y-axis weights
                for wten in (wA, wB):
                    wy = ap3(wten[:], 0, [[K2, NCHUNK], [1, K2]])
                    nc.vector.tensor_tensor(out=wy, in0=wy, in1=m_v,
                                            op=ALU.mult)

                # wq[p, g, k, xj, yj] = wx[xj] * wy[yj]
                wq = pp.tile([128, NCHUNK, K2, 2, 2], f32)
                for xj, wxt in ((0, wA), (1, wB)):
                    for yj, wyt in ((0, wA), (1, wB)):
                        ov = ap3(wq[:], xj * 2 + yj,
                                 [[36, NCHUNK], [4, K2]])
                        yv = ap3(wyt[:], 0, [[K2, NCHUNK], [1, K2]])
                        xv = ap3(wxt[:], NCHUNK * K2, [[K2, NCHUNK], [1, K2]])
                        nc.vector.tensor_tensor(out=ov, in0=yv, in1=xv,
                                                op=ALU.mult)

                in_d = bass.AP(wq[:].tensor, wq[:].offset,
                               [wq[:].ap[0], [1, NCHUNK * 36], [0, 2]])
                out_d = bass.AP(wq2[:].tensor, wq2[:].offset,
                                [wq2[:].ap[0], [2, NCHUNK * 36], [1, 2]])
                nc.vector.tensor_copy(out=out_d, in_=in_d)

                idxf = pp.tile([128, NCHUNK, K2], f32)
                idx_pm = pp.tile([128, GK], i16)
                nc.vector.tensor_scalar(
                    out=idxf[:],
                    in0=ap3(ecl[:], 0, [[K2, NCHUNK], [1, K2]]),
                    scalar1=float(W), scalar2=None, op0=ALU.mult)
                nc.vector.tensor_tensor(
                    out=idxf[:], in0=idxf[:],
                    in1=ap3(ecl[:], NCHUNK * K2, [[K2, NCHUNK], [1, K2]]),
                    op=ALU.add)
                nc.vector.tensor_copy(out=idx_pm[:], in_=idxf[:])

                nc.sync.dma_start(
                    bass.AP(idx_dram, 0, [[1, 128], [128, GK]]),
                    bass.AP(idx_pm[:].tensor, idx_pm[:].offset,
                            [[GK, 128], [1, GK]]),
                )

            for rep in range(8):
                nc.sync.dma_start(
                    idx_wrap[rep * 16:(rep + 1) * 16, :],
                    bass.AP(idx_dram, 0, [[1, 16], [16, NIDX // 16]]),
                )

            # ---------- stages 4-6 ----------
            tab_ap = bass.AP(t_dram, 0, [[2 * CIN, N_ENT], [1, 4 * CIN]])
            with tc.tile_pool(name="gth", bufs=2) as gp, \
                 tc.tile_pool(name="cmb", bufs=6) as cp, \
                 tc.tile_pool(name="gyt", bufs=2) as yp, \
                 tc.tile_pool(name="trp", bufs=3, space="PSUM") as prp, \
                 tc.tile_pool(name="acp", bufs=2, space="PSUM") as acp:
                for blk in range(NBLK):
                    g = gp.tile([128, BPC * K2, 4 * CIN], bft)
                    nc.gpsimd.dma_gather(
                        out_ap=g[:],
                        in_ap=tab_ap,
                        idxs_ap=idx_wrap[:, blk * (BIDX // 16):
                                         (blk + 1) * (BIDX // 16)],
                        num_idxs=BIDX,
                        num_idxs_reg=BIDX,
                        elem_size=4 * CIN,
                        elem_step=2 * CIN,
                        single_packet=False,
                    )
                    gys = []
                    for c in range(BPC):
                        ch = blk * BPC + c
                        gw = cp.tile([128, K2 * 4 * CIN], bft, tag="gw")
                        g_ap = g[:]
                        in0 = bass.AP(g_ap.tensor,
                                      g_ap.offset + c * K2 * 4 * CIN,
                                      [g_ap.ap[0], [CIN, 4 * K2],
                                       [2, CIN // 2], [1, 2]])
                        w_ap = wq2[:]
                        in1 = bass.AP(w_ap.tensor, w_ap.offset + ch * 72,
                                      [w_ap.ap[0], [2, 4 * K2],
                                       [0, CIN // 2], [1, 2]])
                        o_ap2 = gw[:]
                        o4 = bass.AP(o_ap2.tensor, o_ap2.offset,
                                     [o_ap2.ap[0], [CIN, 4 * K2],
                                      [2, CIN // 2], [1, 2]])
                        nc.vector.tensor_tensor(out=o4, in0=in0, in1=in1,
                                                op=ALU.mult)
                        gy = cp.tile([128, K2 * 2 * CIN], bft, tag="gy")
                        a0 = bass.AP(o_ap2.tensor, o_ap2.offset,
                                     [o_ap2.ap[0], [4 * CIN, K2],
                                      [1, 2 * CIN]])
                        a1 = bass.AP(o_ap2.tensor, o_ap2.offset + 2 * CIN,
                                     [o_ap2.ap[0], [4 * CIN, K2],
                                      [1, 2 * CIN]])
                        nc.vector.tensor_tensor(out=gy[:], in0=a0, in1=a1,
                                                op=ALU.add)
                        gys.append(gy)
                    gyt = yp.tile([128, K2, BPC * 128], bft)
                    for s in range(K2):
                        pst = prp.tile([128, BPC * 128], bft)
                        for c in range(BPC):
                            nc.tensor.transpose(
                                pst[:, c * 128:(c + 1) * 128],
                                gys[c][:, s * 128:(s + 1) * 128], ident[:])
                        nc.scalar.copy(gyt[:, s, :], pst[:])
                    acc = acp.tile([OC, BPC * 128], f32)
                    for s in range(K2):
                        nc.tensor.matmul(
                            acc[:], wsm2[:, s, :], gyt[:, s, :],
                            start=(s == 0), stop=(s == K2 - 1))
                    nc.scalar.copy(
                        out_sb[:, blk * BPC * 128:(blk + 1) * BPC * 128],
                        acc[:])
            nc.sync.dma_start(out_t[:], out_sb[:])

    nc.compile()
    return nc


def _host_inputs(x, offset, mask, weight):
    B = x.shape[0]
    w = np.exp(weight - weight.max(axis=2, keepdims=True))
    wsm = (w / w.sum(axis=2, keepdims=True)).astype(np.float32)
    wsm2 = np.transpose(wsm, (2, 1, 0))                      # [k, c, oc]
    wsm2 = np.broadcast_to(wsm2[:, None, :, :], (K2, 2, CIN, OC))
    # device layout [128 (yj,c), K2, OC]
    wsm2 = np.ascontiguousarray(
        np.transpose(wsm2.reshape(K2, 128, OC), (1, 0, 2))
        .reshape(128, K2 * OC).astype(bf16))

    kh = (np.arange(K2) // K).astype(np.float32)
    kw = (np.arange(K2) % K).astype(np.float32)
    cc = np.arange(W, dtype=np.float32)[None, :].repeat(OUT_ROWS, 0).reshape(NP)

    in_maps, meta = [], []
    for b in range(B):
        for h in range(2):
            lo = 0 if h == 0 else H - BAND
            out_lo = 0 if h == 0 else H - OUT_ROWS
            xband = np.zeros((CIN, XCOLS), np.float32)
            xband[:] = x[b, :, lo:lo + BAND, :].reshape(CIN, XCOLS)

            osl = offset[b, :, out_lo:out_lo + OUT_ROWS, :].reshape(18, NP)
            msl = mask[b, :, out_lo:out_lo + OUT_ROWS, :].reshape(K2, NP)
            # axis-blocked: [dy(9), dx(9), m(9)]
            comb = np.concatenate([osl[0::2], osl[1::2], msl], axis=0)  # [27,NP]
            offm = np.transpose(comb.reshape(27, NCHUNK, 128), (2, 1, 0))

            rr = (out_lo + np.arange(OUT_ROWS, dtype=np.float32))[:, None] \
                .repeat(W, 1).reshape(NP)
            by = rr[:, None] - 1 + kh[None, :] + 16.0          # [NP, 9]
            bx = cc[:, None] - 1 + kw[None, :] + 16.0
            byx = np.concatenate([by, bx], axis=1)             # [NP, 18]
            byx = np.transpose(byx.reshape(NCHUNK, 128, 18), (1, 0, 2))

            clampb = np.zeros((128, 2), np.float32)
            clampb[:, 0] = lo + 16.0
            clampb[:, 1] = 16.0

            in_maps.append({
                "xband": np.ascontiguousarray(xband),
                "offm": np.ascontiguousarray(
                    offm.reshape(128, NCHUNK * 27).astype(np.float32)),
                "byx": np.ascontiguousarray(
                    byx.reshape(128, NCHUNK * 18).astype(np.float32)),
                "clampb": clampb,
                "wsm2": wsm2,
            })
            meta.append((b, out_lo))
    return in_maps, meta


def kernel(x, offset, mask, weight):
    x = np.asarray(x, dtype=np.float32)
    offset = np.asarray(offset, dtype=np.float32)
    mask = np.asarray(mask, dtype=np.float32)
    weight = np.asarray(weight, dtype=np.float32)

    if "nc" not in _CACHE:
        _CACHE["nc"] = _build_program()
    nc = _CACHE["nc"]

    in_maps, meta = _host_inputs(x, offset, mask, weight)
    trace = os.environ.get("DEFORM_TRACE", "0") == "1"
    res = run_bass_kernel_spmd(nc, in_maps, core_ids=list(range(8)),
                               trace=trace)
    LAST_RESULTS["exec_time_ns"] = res.exec_time_ns
    LAST_RESULTS["mean_exec_time_ns"] = res.mean_exec_time_ns

    B = x.shape[0]
    out = np.zeros((B, OC, H, W), np.float32)
    for i, (b, out_lo) in enumerate(meta):
        out[b, :, out_lo:out_lo + OUT_ROWS, :] = \
            res.results[i]["out"].reshape(OC, OUT_ROWS, W)
    return out


# revision 15
# speedup vs baseline: 3.0187x; 3.0187x over previous
"""Modulated deformable conv (warp-norm softmax weights) on 8 TRN2 NeuronCores.

Sharding: 8 cores = (batch 4) x (image half 2). Each core gets a 96-row band
of x (16-row halo), its 80 output rows' offsets/masks (host-transposed to
pixel-major), and computes out[64, 80, 160] f32.

Per-core device pipeline:
  1. x band f32 -> bf16 (cast DMA); PE-transposes build a row-pair table
     T[e] = [xT[e] | xT[e+160]] (bf16, 256B entries), staged to DRAM.
  2. Pixel-major index/bilinear-weight prep on DVE. Clamping is slot-remapped:
     one int16 entry index + 4 slot weights reproduce the reference's
     independently-clipped, validity-zeroed corners exactly.
  3. idx DRAM round-trip into dma_gather's 16-partition-wrapped layout.
  4. dma_gather (HBM source, elem_step=128 elems: overlapping entries, so one
     512B read = the full 2x2 corner quad; pixel-major output).
  5. DVE: Gw = G * wq (broadcast AP, 2x mode); Gy = Gw[xj=0] + Gw[xj=1].
  6. PE: transpose Gy slices to (k,yj,c)-major; GEMM vs softmaxed weights
     (yj-replicated) accumulating in PSUM; ACT evacuates.
"""
import os
import sys

sys.path.insert(0, "/opt/trn_rl_repo")

import numpy as np
import ml_dtypes

import concourse.bass as bass
import concourse.bacc as bacc
import concourse.mybir as mybir
from concourse.tile import TileContext
from concourse.masks import make_identity
from concourse.bass_utils import run_bass_kernel_spmd

bf16 = ml_dtypes.bfloat16
f32 = mybir.dt.float32
bft = mybir.dt.bfloat16
i16 = mybir.dt.int16

H = W = 160
CIN = OC = 64
K = 3
K2 = 9
BAND = 96
OUT_ROWS = 80
NP = OUT_ROWS * W          # 12800
NCHUNK = NP // 128         # 100
GK = NCHUNK * K2           # 900
NIDX = NP * K2             # 115200
N_ENT = (BAND - 1) * W     # 15200
XCOLS = BAND * W           # 15360
XPAD = 15488               # transpose-friendly padded width
BPC = 4                    # chunks per gather block
NBLK = NCHUNK // BPC       # 25
BIDX = BPC * K2 * 128      # 4608

ALU = mybir.AluOpType

_CACHE = {}
LAST_RESULTS = {}


def ap3(tile_ap, off, dims):
    return bass.AP(tile_ap.tensor, tile_ap.offset + off,
                   [tile_ap.ap[0]] + dims)


def _build_program():
    nc = bacc.Bacc("TRN2", num_devices=8)

    xb_in = nc.dram_tensor("xband", [CIN, XCOLS], f32, kind="ExternalInput")
    offm_in = nc.dram_tensor("offm", [128, NCHUNK * 27], f32, kind="ExternalInput")
    byx_in = nc.dram_tensor("byx", [128, NCHUNK * 18], f32, kind="ExternalInput")
    clampb_in = nc.dram_tensor("clampb", [128, 2], f32, kind="ExternalInput")
    wsm2_in = nc.dram_tensor("wsm2", [128, K2 * OC], bft, kind="ExternalInput")
    permf_in = nc.dram_tensor("permf", [128, 128], f32, kind="ExternalInput")
    out_t = nc.dram_tensor("out", [OC, NP], f32, kind="ExternalOutput")
    t_dram = nc.dram_tensor("tdram", [XPAD + 1, 2 * CIN], bft, kind="Internal")

    with TileContext(nc) as tc:
        with tc.tile_pool(name="const", bufs=1) as cpool:
            ident = cpool.tile([128, 128], bft)
            make_identity(nc, ident[:])
            permf = cpool.tile([128, 128], f32)
            nc.sync.dma_start(permf[:], permf_in[:])
            wsm2 = cpool.tile([128, K2, OC], bft)
            nc.sync.dma_start(wsm2[:], wsm2_in[:])
            clampb = cpool.tile([128, 2], f32)
            nc.sync.dma_start(clampb[:], clampb_in[:])
            wq2 = cpool.tile([128, NCHUNK, 36, 2], bft)
            idx_wrap = cpool.tile([128, NIDX // 16], i16)
            out_sb = cpool.tile([OC, NP], f32)

            # ---------- stage 1: quad table ----------
            with tc.tile_pool(name="tbl", bufs=1) as tp, \
                 tc.tile_pool(name="tpsum", bufs=3, space="PSUM") as tpp:
                xb = tp.tile([CIN, XPAD], bft)
                nc.vector.memset(xb[:, XCOLS:], 0.0)
                nc.gpsimd.dma_start(xb[:, :XCOLS], xb_in[:])   # casts f32->bf16
                tsb = tp.tile([128, 119, 2, CIN], bft)
                for q in range(0, 119, 4):
                    nq = min(4, 119 - q)
                    ps = tpp.tile([128, 4, 2, CIN], bft)
                    for j in range(nq):
                        t = q + j
                        nc.tensor.transpose(
                            ps[:, j, 0, :], xb[:, t * 128:(t + 1) * 128],
                            ident[:CIN, :CIN])
                        nc.tensor.transpose(
                            ps[:, j, 1, :], xb[:, t * 128 + W:t * 128 + W + 128],
                            ident[:CIN, :CIN])
                    nc.scalar.copy(tsb[:, q:q + nq, :, :], ps[:, :nq, :, :])
                nc.sync.dma_start(
                    bass.AP(t_dram, 0,
                            [[2 * CIN, 128], [128 * 2 * CIN, 119],
                             [1, 2 * CIN]]),
                    bass.AP(tsb[:].tensor, tsb[:].offset,
                            [[119 * 2 * CIN, 128], [2 * CIN, 119], [1, 2 * CIN]]),
                )

            # ---------- stage 2: prep ----------
            with tc.tile_pool(name="prep", bufs=1) as pp:
                offm = pp.tile([128, NCHUNK, 27], f32)
                nc.sync.dma_start(offm[:], offm_in[:])
                byx = pp.tile([128, NCHUNK, 18], f32)
                nc.sync.dma_start(byx[:], byx_in[:])

                o_ap = offm[:]
                dyx = ap3(o_ap, 0, [[27, NCHUNK], [1, 18]])
                m_v = ap3(o_ap, 18, [[27, NCHUNK], [1, K2]])

                pyx = pp.tile([128, NCHUNK, 18], f32)
                pyx_v = ap3(pyx[:], 0, [[18, NCHUNK], [1, 18]])
                nc.vector.tensor_tensor(out=pyx_v, in0=dyx,
                                        in1=ap3(byx[:], 0,
                                                [[18, NCHUNK], [1, 18]]),
                                        op=ALU.add)
                yx0 = pp.tile([128, NCHUNK, 18], f32)   # floor + 16 bias
                # floor via round-to-nearest(py - 0.5) using the 2^23 trick;
                # exact-integer py lands on (y0-1, fy=1) which samples
                # identically.
                nc.vector.tensor_scalar(out=yx0[:], in0=pyx[:],
                                        scalar1=float(2 ** 23) - 0.5,
                                        scalar2=float(2 ** 23),
                                        op0=ALU.add, op1=ALU.subtract)
                fyx = pp.tile([128, NCHUNK, 18], f32)
                nc.vector.tensor_tensor(out=fyx[:], in0=pyx[:], in1=yx0[:],
                                        op=ALU.subtract)

                wA = pp.tile([128, 2, NCHUNK, K2], f32)
                wB = pp.tile([128, 2, NCHUNK, K2], f32)
                ecl = pp.tile([128, 2, NCHUNK, K2], f32)
                tmp1 = pp.tile([128, NCHUNK, K2], f32)
                tmp2 = pp.tile([128, NCHUNK, K2], f32)
                tmp3 = pp.tile([128, NCHUNK, K2], f32)
                gv0 = pp.tile([128, NCHUNK, K2], f32)
                gv1 = pp.tile([128, NCHUNK, K2], f32)

                for ax in range(2):
                    v0 = ap3(yx0[:], ax * K2, [[18, NCHUNK], [1, K2]])
                    f0 = ap3(fyx[:], ax * K2, [[18, NCHUNK], [1, K2]])
                    eo = ap3(ecl[:], ax * NCHUNK * K2, [[K2, NCHUNK], [1, K2]])
                    wAo = ap3(wA[:], ax * NCHUNK * K2, [[K2, NCHUNK], [1, K2]])
                    wBo = ap3(wB[:], ax * NCHUNK * K2, [[K2, NCHUNK], [1, K2]])
                    emax = 94.0 if ax == 0 else 158.0
                    # t = yx0 - clamp_bias (per-core AP scalar)
                    nc.vector.tensor_scalar(out=tmp1[:], in0=v0,
                                            scalar1=clampb[:, ax:ax + 1],
                                            scalar2=None, op0=ALU.subtract)
                    nc.vector.tensor_scalar(out=eo, in0=tmp1[:], scalar1=0.0,
                                            scalar2=emax, op0=ALU.max,
                                            op1=ALU.min)
                    nc.vector.tensor_tensor(out=tmp2[:], in0=tmp1[:], in1=eo,
                                            op=ALU.subtract)
                    # validity is implied by the eq-gates: every corner
                    # passing an eq test lies in the valid image range.
                    # gv0 = 1 - f
                    nc.vector.tensor_scalar(out=gv0[:], in0=f0, scalar1=1.0,
                                            scalar2=-1.0, op0=ALU.subtract,
                                            op1=ALU.mult)
                    # wA = (te==0)*(1-f) + (te==-1)*f
                    nc.vector.tensor_scalar(out=tmp1[:], in0=tmp2[:],
                                            scalar1=0.0, scalar2=None,
                                            op0=ALU.is_equal)
                    nc.vector.tensor_tensor(out=tmp1[:], in0=tmp1[:],
                                            in1=gv0[:], op=ALU.mult)
                    nc.vector.tensor_scalar(out=tmp3[:], in0=tmp2[:],
                                            scalar1=-1.0, scalar2=None,
                                            op0=ALU.is_equal)
                    nc.vector.tensor_tensor(out=tmp3[:], in0=tmp3[:],
                                            in1=f0, op=ALU.mult)
                    nc.vector.tensor_tensor(out=wAo, in0=tmp1[:], in1=tmp3[:],
                                            op=ALU.add)
                    # wB = (te==0)*f + (te==1)*(1-f)
                    nc.vector.tensor_scalar(out=tmp1[:], in0=tmp2[:],
                                            scalar1=0.0, scalar2=None,
                                            op0=ALU.is_equal)
                    nc.vector.tensor_tensor(out=tmp1[:], in0=tmp1[:],
                                            in1=f0, op=ALU.mult)
                    nc.vector.tensor_scalar(out=tmp3[:], in0=tmp2[:],
                                            scalar1=1.0, scalar2=None,
                                            op0=ALU.is_equal)
                    nc.vector.tensor_tensor(out=tmp3[:], in0=tmp3[:],
                                            in1=gv0[:], op=ALU.mult)
                    nc.vector.tensor_tensor(out=wBo, in0=tmp1[:], in1=tmp3[:],
                                            op=ALU.add)

                # fold mask into y-axis weights
                for wten in (wA, wB):
                    wy = ap3(wten[:], 0, [[K2, NCHUNK], [1, K2]])
                    nc.vector.tensor_tensor(out=wy, in0=wy, in1=m_v,
                                            op=ALU.mult)

                # wq[p, g, k, xj, yj] = wx[xj] * wy[yj]
                wq = pp.tile([128, NCHUNK, K2, 2, 2], f32)
                for xj, wxt in ((0, wA), (1, wB)):
                    for yj, wyt in ((0, wA), (1, wB)):
                        ov = ap3(wq[:], xj * 2 + yj,
                                 [[36, NCHUNK], [4, K2]])
                        yv = ap3(wyt[:], 0, [[K2, NCHUNK], [1, K2]])
                        xv = ap3(wxt[:], NCHUNK * K2, [[K2, NCHUNK], [1, K2]])
                        nc.vector.tensor_tensor(out=ov, in0=yv, in1=xv,
                                                op=ALU.mult)

                in_d = bass.AP(wq[:].tensor, wq[:].offset,
                               [wq[:].ap[0], [1, NCHUNK * 36], [0, 2]])
                out_d = bass.AP(wq2[:].tensor, wq2[:].offset,
                                [wq2[:].ap[0], [2, NCHUNK * 36], [1, 2]])
                nc.vector.tensor_copy(out=out_d, in_=in_d)

                idxf = pp.tile([128, NCHUNK, K2], f32)
                nc.vector.tensor_scalar(
                    out=idxf[:],
                    in0=ap3(ecl[:], 0, [[K2, NCHUNK], [1, K2]]),
                    scalar1=float(W), scalar2=None, op0=ALU.mult)
                nc.vector.tensor_tensor(
                    out=idxf[:], in0=idxf[:],
                    in1=ap3(ecl[:], NCHUNK * K2, [[K2, NCHUNK], [1, K2]]),
                    op=ALU.add)
                # matmul-transpose idx (f32, exact) against a lane
                # permutation: A[gk', (q,d)] = idx[16d+q, gk'] so both sides
                # of the 16-wrap DMA move 16B-contiguous runs of 8 samples.
                asb = pp.tile([128, 8, 128], i16)
                idxf_ap = idxf[:]
                with tc.tile_pool(name="idxp", bufs=2, space="PSUM") as ipp:
                    for t in range(8):
                        ncols = 128 if t < 7 else GK - 7 * 128
                        psi = ipp.tile([128, 128], f32)
                        tin = bass.AP(idxf_ap.tensor,
                                      idxf_ap.offset + t * 128,
                                      [idxf_ap.ap[0], [1, ncols]])
                        nc.tensor.matmul(psi[:ncols, :], tin, permf[:],
                                         start=True, stop=True)
                        nc.scalar.copy(asb[:ncols, t, :], psi[:ncols, :])
                # wrap[q, (t*128+gkp)*8 + d] = A[gkp, t, q*8+d]
                a_ap = asb[:]
                for q in range(16):
                    eng = nc.sync if q % 2 == 0 else nc.scalar
                    for t in range(7):
                        wslice = idx_wrap[q:q + 1, :]
                        eng.dma_start(
                            bass.AP(wslice.tensor, wslice.offset + t * 1024,
                                    [wslice.ap[0], [8, 128], [1, 8]]),
                            bass.AP(a_ap.tensor,
                                    a_ap.offset + t * 128 + q * 8,
                                    [a_ap.ap[0], [1, 8]]),
                        )
                    tail = asb[0:4, :, :]
                    wslice = idx_wrap[q:q + 1, :]
                    eng.dma_start(
                        bass.AP(wslice.tensor, wslice.offset + 7168,
                                [wslice.ap[0], [8, 4], [1, 8]]),
                        bass.AP(tail.tensor, tail.offset + 7 * 128 + q * 8,
                                [tail.ap[0], [1, 8]]),
                    )
            for rep in range(1, 8):
                nc.sync.dma_start(
                    idx_wrap[rep * 16:(rep + 1) * 16, :],
                    idx_wrap[0:16, :],
                )

            # ---------- stages 4-6 ----------
            tab_ap = bass.AP(t_dram, 0, [[2 * CIN, N_ENT], [1, 4 * CIN]])
            with tc.tile_pool(name="gth", bufs=2) as gp, \
                 tc.tile_pool(name="cmb", bufs=6) as cp, \
                 tc.tile_pool(name="gyt", bufs=2) as yp, \
                 tc.tile_pool(name="trp", bufs=3, space="PSUM") as prp, \
                 tc.tile_pool(name="acp", bufs=2, space="PSUM") as acp:
                for blk in range(NBLK):
                    g = gp.tile([128, BPC * K2, 4 * CIN], bft)
                    nc.gpsimd.dma_gather(
                        out_ap=g[:],
                        in_ap=tab_ap,
                        idxs_ap=idx_wrap[:, blk * (BIDX // 16):
                                         (blk + 1) * (BIDX // 16)],
                        num_idxs=BIDX,
                        num_idxs_reg=BIDX,
                        elem_size=4 * CIN,
                        elem_step=2 * CIN,
                        single_packet=False,
                    )
                    gys = []
                    for c in range(BPC):
                        ch = blk * BPC + c
                        gw = cp.tile([128, K2 * 4 * CIN], bft, tag="gw")
                        g_ap = g[:]
                        in0 = bass.AP(g_ap.tensor,
                                      g_ap.offset + c * K2 * 4 * CIN,
                                      [g_ap.ap[0], [CIN, 4 * K2],
                                       [2, CIN // 2], [1, 2]])
                        w_ap = wq2[:]
                        in1 = bass.AP(w_ap.tensor, w_ap.offset + ch * 72,
                                      [w_ap.ap[0], [2, 4 * K2],
                                       [0, CIN // 2], [1, 2]])
                        o_ap2 = gw[:]
                        o4 = bass.AP(o_ap2.tensor, o_ap2.offset,
                                     [o_ap2.ap[0], [CIN, 4 * K2],
                                      [2, CIN // 2], [1, 2]])
                        nc.vector.tensor_tensor(out=o4, in0=in0, in1=in1,
                                                op=ALU.mult)
                        gy = cp.tile([128, K2 * 2 * CIN], bft, tag="gy")
                        a0 = bass.AP(o_ap2.tensor, o_ap2.offset,
                                     [o_ap2.ap[0], [4 * CIN, K2],
                                      [1, 2 * CIN]])
                        a1 = bass.AP(o_ap2.tensor, o_ap2.offset + 2 * CIN,
                                     [o_ap2.ap[0], [4 * CIN, K2],
                                      [1, 2 * CIN]])
                        nc.vector.tensor_tensor(out=gy[:], in0=a0, in1=a1,
                                                op=ALU.add)
                        gys.append(gy)
                    gyt = yp.tile([128, K2, BPC * 128], bft)
                    for s in range(K2):
                        pst = prp.tile([128, BPC * 128], bft)
                        for c in range(BPC):
                            nc.tensor.transpose(
                                pst[:, c * 128:(c + 1) * 128],
                                gys[c][:, s * 128:(s + 1) * 128], ident[:])
                        nc.scalar.copy(gyt[:, s, :], pst[:])
                    acc = acp.tile([OC, BPC * 128], f32)
                    for s in range(K2):
                        nc.tensor.matmul(
                            acc[:], wsm2[:, s, :], gyt[:, s, :],
                            start=(s == 0), stop=(s == K2 - 1))
                    nc.scalar.copy(
                        out_sb[:, blk * BPC * 128:(blk + 1) * BPC * 128],
                        acc[:])
            nc.sync.dma_start(out_t[:], out_sb[:])

    nc.compile()
    return nc


def _host_inputs(x, offset, mask, weight):
    B = x.shape[0]
    w = np.exp(weight - weight.max(axis=2, keepdims=True))
    wsm = (w / w.sum(axis=2, keepdims=True)).astype(np.float32)
    wsm2 = np.transpose(wsm, (2, 1, 0))                      # [k, c, oc]
    wsm2 = np.broadcast_to(wsm2[:, None, :, :], (K2, 2, CIN, OC))
    # device layout [128 (yj,c), K2, OC]
    wsm2 = np.ascontiguousarray(
        np.transpose(wsm2.reshape(K2, 128, OC), (1, 0, 2))
        .reshape(128, K2 * OC).astype(bf16))

    kh = (np.arange(K2) // K).astype(np.float32)
    kw = (np.arange(K2) % K).astype(np.float32)
    cc = np.arange(W, dtype=np.float32)[None, :].repeat(OUT_ROWS, 0).reshape(NP)

    in_maps, meta = [], []
    for b in range(B):
        for h in range(2):
            lo = 0 if h == 0 else H - BAND
            out_lo = 0 if h == 0 else H - OUT_ROWS
            xband = np.zeros((CIN, XCOLS), np.float32)
            xband[:] = x[b, :, lo:lo + BAND, :].reshape(CIN, XCOLS)

            osl = offset[b, :, out_lo:out_lo + OUT_ROWS, :].reshape(18, NP)
            msl = mask[b, :, out_lo:out_lo + OUT_ROWS, :].reshape(K2, NP)
            # axis-blocked: [dy(9), dx(9), m(9)]
            comb = np.concatenate([osl[0::2], osl[1::2], msl], axis=0)  # [27,NP]
            offm = np.transpose(comb.reshape(27, NCHUNK, 128), (2, 1, 0))

            rr = (out_lo + np.arange(OUT_ROWS, dtype=np.float32))[:, None] \
                .repeat(W, 1).reshape(NP)
            by = rr[:, None] - 1 + kh[None, :] + 16.0          # [NP, 9]
            bx = cc[:, None] - 1 + kw[None, :] + 16.0
            byx = np.concatenate([by, bx], axis=1)             # [NP, 18]
            byx = np.transpose(byx.reshape(NCHUNK, 128, 18), (1, 0, 2))

            clampb = np.zeros((128, 2), np.float32)
            clampb[:, 0] = lo + 16.0
            clampb[:, 1] = 16.0

            permf = np.zeros((128, 128), np.float32)
            for p in range(128):
                permf[p, (p % 16) * 8 + p // 16] = 1.0

            in_maps.append({
                "xband": np.ascontiguousarray(xband),
                "permf": permf,
                "offm": np.ascontiguousarray(
                    offm.reshape(128, NCHUNK * 27).astype(np.float32)),
                "byx": np.ascontiguousarray(
                    byx.reshape(128, NCHUNK * 18).astype(np.float32)),
                "clampb": clampb,
                "wsm2": wsm2,
            })
            meta.append((b, out_lo))
    return in_maps, meta


def kernel(x, offset, mask, weight):
    x = np.asarray(x, dtype=np.float32)
    offset = np.asarray(offset, dtype=np.float32)
    mask = np.asarray(mask, dtype=np.float32)
    weight = np.asarray(weight, dtype=np.float32)

    if "nc" not in _CACHE:
        _CACHE["nc"] = _build_program()
    nc = _CACHE["nc"]

    in_maps, meta = _host_inputs(x, offset, mask, weight)
    trace = os.environ.get("DEFORM_TRACE", "0") == "1"
    res = run_bass_kernel_spmd(nc, in_maps, core_ids=list(range(8)),
                               trace=trace)
    LAST_RESULTS["exec_time_ns"] = res.exec_time_ns
    LAST_RESULTS["mean_exec_time_ns"] = res.mean_exec_time_ns

    B = x.shape[0]
    out = np.zeros((B, OC, H, W), np.float32)
    for i, (b, out_lo) in enumerate(meta):
        out[b, :, out_lo:out_lo + OUT_ROWS, :] = \
            res.results[i]["out"].reshape(OC, OUT_ROWS, W)
    return out
